# revision 15
# baseline (speedup 1.0000x reference)
"""Trainium2 Bass kernel: full encoder-decoder transformer decoder layer.

Contract: kernel(**inputs) takes FULL unsharded inputs and returns the
FULL [B, T, D] float32 output.

Sharding: data-parallel over (batch, T-half) -> 8 cores, zero
collectives.  Each core computes its TL=1024 decoder rows end-to-end;
full-T K/V projections are computed redundantly by the 2 cores sharing
a batch element.

v3: the whole layer is one explicit software pipeline.  The engines
execute their queues in order, so independent projection work is woven
into the exp-bound attention loops at matmul granularity via filler
generators (cross-K/V during self-attn, out-proj/LN/next-Q during the
following attention chunk).  SBUF is managed with two allocation stacks
and heavy aliasing: the merged attention output overwrites Q, the
pre-LN activations overwrite K, the LN output overwrites Q again.
"""

from collections import deque
from contextlib import ExitStack

import ml_dtypes
import numpy as np

import concourse.bass as bass
import concourse.mybir as mybir
import concourse.tile as tile
from concourse import bacc
from concourse.bass_utils import run_bass_kernel_spmd

P = 128
HD = 64  # head dim (fixed)
BF = mybir.dt.bfloat16
F32 = mybir.dt.float32
AF = mybir.ActivationFunctionType
ALU = mybir.AluOpType
EPS = 1e-5


def build_program(D=1024, H=16, T=2048, TL=1024, S=2048, DFF=4096, loop_n=1):
    assert D == H * HD
    KT = D // P
    NKT = T // P
    NSK = S // P
    FT = DFF // P
    HP = H // 2
    assert HP == KT
    QC = min(512, TL)
    NQ = TL // QC

    nc = bacc.Bacc()
    tens = {}

    def din(name, shape, dtype=BF):
        tens[name] = nc.declare_dram_parameter(name, list(shape), dtype,
                                               isOutput=False)
        return tens[name]

    din("xdT", (KT, P, T))
    din("xres", (KT, P, TL))
    din("xeT", (KT, P, S))
    for nm in ("wq", "wk", "wv", "wo1", "wqc", "wkc", "wvc", "wo2"):
        din(nm, (KT, P, D))
    din("w1", (KT, P, DFF))
    din("w2", (FT, P, D))
    din("ppb", (P, 13 * KT + FT), F32)
    din("bv_row", (1, D), F32)
    din("bvc_row", (1, D), F32)

    tens["outT"] = nc.declare_dram_parameter("outT", [KT, P, TL], BF,
                                             isOutput=True)

    tens["res1_spill"] = nc.dram_tensor("res1_spill", [KT, P, TL], BF)
    tens["res2_spill"] = nc.dram_tensor("res2_spill", [KT, P, TL], BF)
    tens["vc_spill"] = nc.dram_tensor("vc_spill", [NSK, P, H * (HD + 1)], BF)
    tens["r_bounce"] = nc.dram_tensor("r_bounce", [2, H, NQ, QC], F32)

    cfg = dict(D=D, H=H, T=T, TL=TL, S=S, DFF=DFF, KT=KT, NKT=NKT,
               NSK=NSK, FT=FT, HP=HP, QC=QC, NQ=NQ, tens=tens)

    with tile.TileContext(nc) as tc:
        if loop_n > 1:
            with tc.For_i(0, loop_n, 1) as _i:
                _build(tc, cfg)
        else:
            _build(tc, cfg)

    nc.finalize()
    return nc


def _build(tc, cfg):
    nc = tc.nc
    D, H, T, TL, S, DFF = (cfg["D"], cfg["H"], cfg["T"], cfg["TL"], cfg["S"],
                           cfg["DFF"])
    KT, NKT, NSK, FT, HP, QC, NQ = (cfg["KT"], cfg["NKT"], cfg["NSK"],
                                    cfg["FT"], cfg["HP"], cfg["QC"], cfg["NQ"])
    tens = cfg["tens"]

    def dram(name):
        return tens[name][:]

    ctx = ExitStack()
    const = ctx.enter_context(tc.tile_pool(name="const", bufs=1))
    stream2 = ctx.enter_context(tc.tile_pool(name="stream2", bufs=4))
    avsp = ctx.enter_context(tc.tile_pool(name="avsp", bufs=2))
    lnp = ctx.enter_context(tc.tile_pool(name="lnp", bufs=1))
    zsqp = ctx.enter_context(tc.tile_pool(name="zsqp", bufs=2))
    smallp = ctx.enter_context(tc.tile_pool(name="smallp", bufs=2))
    stagep = ctx.enter_context(tc.tile_pool(name="stagep", bufs=2))
    acc = ctx.enter_context(tc.tile_pool(name="acc", bufs=2, space="PSUM"))
    avp = ctx.enter_context(tc.tile_pool(name="avp", bufs=1, space="PSUM"))
    scp = ctx.enter_context(tc.tile_pool(name="scp", bufs=2, space="PSUM"))

    # ---------------- constants ----------------
    ones_bf = const.tile([P, P], BF, tag="ones_bf", name="ones_bf")
    nc.vector.memset(ones_bf[:], 1.0)
    eps_t = const.tile([P, 1], F32, tag="eps_t", name="eps_t")
    nc.vector.memset(eps_t[:], EPS)

    ppb = const.tile([P, 13 * KT + FT], F32, tag="ppb", name="ppb")
    nc.sync.dma_start(out=ppb[:], in_=dram("ppb"))
    _ppi = [0]

    def pp_slice(n):
        o = _ppi[0]
        _ppi[0] += n
        return [ppb[:, o + j:o + j + 1] for j in range(n)]

    bq = pp_slice(KT); bk = pp_slice(KT)
    bo1 = pp_slice(KT); bqc = pp_slice(KT)
    bkc = pp_slice(KT); bo2 = pp_slice(KT)
    b2f = pp_slice(KT)
    g1 = pp_slice(KT); be1 = pp_slice(KT)
    g2 = pp_slice(KT); be2 = pp_slice(KT)
    g3 = pp_slice(KT); be3 = pp_slice(KT)
    b1f = pp_slice(FT)

    def bias_bcast(name):
        tl_ = const.tile([P, D], BF, tag=f"{name}_bc", name=f"{name}_bc")
        src = dram(name)
        bcast_ap = bass.AP(tensor=src.tensor, offset=0, ap=[[0, P], [1, D]])
        nc.gpsimd.dma_start(out=tl_[:], in_=bcast_ap)
        return tl_

    vb_bc = bias_bcast("bv_row")
    vcb_bc = bias_bcast("bvc_row")

    # ---------------- filler weave ----------------
    fillers = deque()

    def pull(n_mms):
        while n_mms > 0 and fillers:
            try:
                n_mms -= next(fillers[0])
            except StopIteration:
                fillers.popleft()

    def drain():
        while fillers:
            try:
                next(fillers[0])
            except StopIteration:
                fillers.popleft()

    def run_gen(g):
        for _ in g:
            pass

    ACCG = 2

    def projT_gen(wname, x_tiles, nF, Tlen, evict, tcis, kt_in=None,
                  wp_bufs=2, pname=None):
        """Transposed projection generator; yields #matmuls after each
        contraction step so it can be woven as attention filler."""
        kt_in = kt_in if kt_in is not None else KT
        C = min(512, Tlen)
        nfj = nF // P
        w = dram(wname)
        pname = pname or f"wp_{wname}{tcis[0]}"
        wr = w.rearrange("k p d -> p k d")
        with tc.tile_pool(name=pname, bufs=wp_bufs) as wp:
            for fg in range((nfj + ACCG - 1) // ACCG):
                js = list(range(fg * ACCG, min((fg + 1) * ACCG, nfj)))
                cw = len(js) * P
                wt_all = wp.tile([P, kt_in * cw], BF, tag="w",
                                 name=f"w_{pname}")
                nc.sync.dma_start(
                    out=wt_all[:],
                    in_=wr[:, :, js[0] * P:(js[-1] + 1) * P])
                wts = [wt_all[:, ki * cw:(ki + 1) * cw]
                       for ki in range(kt_in)]
                for tci in tcis:
                    ps = [acc.tile([P, C], F32, tag="acc", name="acc_ps")
                          for _ in js]
                    for ki in range(kt_in):
                        for jj in range(len(js)):
                            nc.tensor.matmul(
                                ps[jj][:],
                                lhsT=wts[ki][:, jj * P:(jj + 1) * P],
                                rhs=x_tiles[ki][:, tci * C:(tci + 1) * C],
                                start=(ki == 0), stop=(ki == kt_in - 1))
                        if ki < kt_in - 1:
                            yield len(js)
                    for jj, fj in enumerate(js):
                        evict(fj, tci, C, ps[jj])
                    yield len(js)

    # ---------------- attention ----------------
    def attention_chunk(Kt, Qt, Vt, nkt, expp, tag, qi, bounce_idx,
                        cad=1, jt_tail=3):
        """One query chunk over all head pairs; merged output overwrites
        Qt.  AV lags scores by 2 kt so exp (ScalarE) stays off the PE
        critical path; fillers are pulled each kt step."""
        rb = tens["r_bounce"]
        qsl = slice(qi * QC, (qi + 1) * QC)
        for jt in range(HP):
            avE = avp.tile([P, QC], F32, tag="avE", name="avE")
            avO = avp.tile([P, QC], F32, tag="avO", name="avO")
            exs = {}

            def scores(kt):
                sc = scp.tile([P, 2 * QC], F32, tag="sc", name="sc_ps")
                ksl = slice(kt * P, (kt + 1) * P)
                nc.tensor.matmul(
                    sc[:, 0:QC],
                    lhsT=Kt[jt][0:HD, ksl], rhs=Qt[jt][0:HD, qsl],
                    start=True, stop=True, tile_position=(0, 0))
                nc.tensor.matmul(
                    sc[:, QC:2 * QC],
                    lhsT=Kt[jt][HD:P, ksl], rhs=Qt[jt][HD:P, qsl],
                    start=True, stop=True, tile_position=(HD, 0))
                ex = expp.tile([P, 2 * QC], BF, tag="ex", name=f"ex_{tag}")
                nc.scalar.activation(ex[:], sc[:], AF.Exp, scale=0.125)
                exs[kt] = ex

            def av(kt):
                ex = exs.pop(kt)
                vsl = Vt[kt][:].rearrange("p (h c) -> p h c", c=HD + 1)
                nc.tensor.matmul(
                    avE[0:HD + 1, :], lhsT=vsl[:, 2 * jt, :],
                    rhs=ex[:, 0:QC],
                    start=(kt == 0), stop=(kt == nkt - 1))
                nc.tensor.matmul(
                    avO[0:HD + 1, :], lhsT=vsl[:, 2 * jt + 1, :],
                    rhs=ex[:, QC:2 * QC],
                    start=(kt == 0), stop=(kt == nkt - 1))

            for kt in range(nkt):
                scores(kt)
                if kt >= 2:
                    av(kt - 2)
                    pull(cad)
            av(nkt - 2)
            av(nkt - 1)
            for av_t, hh in ((avE, 2 * jt), (avO, 2 * jt + 1)):
                b0 = (hh % 2) * HD
                avs = avsp.tile([HD + 1, QC], F32, tag="avs", name="avs")
                nc.vector.tensor_copy(out=avs[:], in_=av_t[0:HD + 1, :])
                nc.vector.reciprocal(avs[HD:HD + 1, :], avs[HD:HD + 1, :])
                rslot = rb[bounce_idx, hh, qi]
                nc.gpsimd.dma_start(out=rslot, in_=avs[HD:HD + 1, :])
                bc = smallp.tile([HD, QC], F32, tag="bcast", name="bcast")
                r_bcast = bass.AP(tensor=rslot.tensor, offset=rslot.offset,
                                  ap=[[0, HD]] + list(rslot.ap[-1:]))
                nc.gpsimd.dma_start(out=bc[:], in_=r_bcast)
                nc.vector.tensor_mul(
                    Qt[jt][b0:b0 + HD, qsl], avs[0:HD, :], bc[:])
            pull(jt_tail)

    # ---------------- layer norm (bf16 z, chunked) ----------------
    def layer_norm_chunk(z_bf, g, be, tci, out_bf, spill=None,
                         out_dram=None):
        inv_d = 1.0 / D
        sl = slice(tci * QC, (tci + 1) * QC)
        psA = acc.tile([P, QC], F32, tag="acc", name="psA")
        psB = acc.tile([P, QC], F32, tag="acc", name="psB")
        for ki in range(KT):
            nc.tensor.matmul(psA[:], lhsT=ones_bf[:], rhs=z_bf[ki][:, sl],
                             start=(ki == 0), stop=(ki == KT - 1))
        for ki in range(KT):
            zq = zsqp.tile([P, QC], BF, tag="zsq", name="zsq")
            nc.vector.tensor_mul(zq[:], z_bf[ki][:, sl], z_bf[ki][:, sl])
            nc.tensor.matmul(psB[:], lhsT=ones_bf[:], rhs=zq[:],
                             start=(ki == 0), stop=(ki == KT - 1))
        mean = lnp.tile([P, QC], F32, tag="mean", name="mean")
        msq = lnp.tile([P, QC], F32, tag="msq", name="msq")
        nc.vector.tensor_scalar_mul(mean[:], psA[:], inv_d)
        nc.vector.tensor_scalar_mul(msq[:], psB[:], inv_d)
        tmp2 = lnp.tile([P, QC], F32, tag="tmp2", name="tmp2")
        nc.vector.tensor_mul(tmp2[:], mean[:], mean[:])
        nc.vector.tensor_sub(msq[:], msq[:], tmp2[:])      # var
        nc.scalar.activation(tmp2[:], msq[:], AF.Sqrt, bias=eps_t[:])
        rstd = msq
        nc.vector.reciprocal(rstd[:], tmp2[:])
        mr = mean
        nc.vector.tensor_mul(mr[:], mean[:], rstd[:])
        for ki in range(KT):
            tmp = lnp.tile([P, QC], F32, tag="lntmp", name="lntmp")
            nc.vector.tensor_mul(tmp[:], z_bf[ki][:, sl], rstd[:])
            nc.vector.tensor_sub(tmp[:], tmp[:], mr[:])
            nc.vector.tensor_scalar(
                out=out_bf[ki][:, sl], in0=tmp[:],
                scalar1=g[ki][:], scalar2=be[ki][:],
                op0=ALU.mult, op1=ALU.add)
            if spill is not None:
                nc.gpsimd.dma_start(out=spill[ki][:, sl],
                                    in_=out_bf[ki][:, sl])
            if out_dram is not None:
                nc.gpsimd.dma_start(out=out_dram[ki][:, sl],
                                    in_=out_bf[ki][:, sl])

    NVJ = D // 512 if D >= 512 else 1
    VC = min(512, D)

    # =====================================================================
    # program
    # =====================================================================

    # left stack: pkq (K^T,Q^T self) | pvst (V self) | expp | px (xdT)
    es_kq = ExitStack()
    pkq = es_kq.enter_context(tc.tile_pool(name="pkq", bufs=1))
    KTt = [pkq.tile([P, T], BF, tag=f"KTt{j}", name=f"KTt{j}")
           for j in range(HP)]
    QTt = [pkq.tile([P, TL], BF, tag=f"QTt{j}", name=f"QTt{j}")
           for j in range(HP)]
    es_vst = ExitStack()
    pvst = es_vst.enter_context(tc.tile_pool(name="pvst", bufs=1))
    Vst = [pvst.tile([P, H * (HD + 1)], BF, tag=f"Vst{k}", name=f"Vst{k}")
           for k in range(NKT)]
    es_exp = ExitStack()
    expp = es_exp.enter_context(tc.tile_pool(name="expp", bufs=3))

    # right stack: pqc (Q^T cross) | pkc (K^T cross)
    es_qc = ExitStack()
    pqc = es_qc.enter_context(tc.tile_pool(name="pqc", bufs=1, side="right"))
    QTc = [pqc.tile([P, TL], BF, tag=f"QTc{j}", name=f"QTc{j}")
           for j in range(HP)]
    es_kc = ExitStack()
    pkc = es_kc.enter_context(tc.tile_pool(name="pkc", bufs=1, side="right"))
    KTc = [pkc.tile([P, S], BF, tag=f"KTc{j}", name=f"KTc{j}")
           for j in range(HP)]

    # aliases (see module docstring)
    z1b = [KTt[k][:, 0:TL] for k in range(KT)]     # pre-LN1 activations
    res_bf = QTt                                    # LN1 output
    z2b = [KTc[k][:, 0:TL] for k in range(KT)]     # pre-LN2 activations
    res2_bf = QTc                                   # LN2 output

    # ---- head: xdT load; self K/Q/V (dense) ----
    es_x = ExitStack()
    px = es_x.enter_context(tc.tile_pool(name="px", bufs=1))
    xdT = []
    for ki in range(KT):
        tl_ = px.tile([P, T], BF, tag=f"xdT{ki}", name=f"xdT{ki}")
        nc.sync.dma_start(out=tl_[:], in_=dram("xdT")[ki])
        xdT.append(tl_)
    xqT = [t[:, 0:TL] for t in xdT]

    def ev_k(fj, tci, C, ps):
        nc.vector.tensor_scalar_add(
            out=KTt[fj][:, tci * C:(tci + 1) * C], in0=ps[:],
            scalar1=bk[fj][:])

    def ev_q(fj, tci, C, ps):
        nc.vector.tensor_scalar_add(
            out=QTt[fj][:, tci * C:(tci + 1) * C], in0=ps[:],
            scalar1=bq[fj][:])

    run_gen(projT_gen("wk", xdT, D, T, ev_k, tcis=list(range(T // 512))))
    run_gen(projT_gen("wq", xqT, D, TL, ev_q, tcis=list(range(NQ))))

    def v_gen(x_from_dram, wvname, nkt, vbias_bc, evict, pname):
        """V projection: kt-outer, weights resident, x streamed [P,P]."""
        wr = dram(wvname).rearrange("k p d -> p k d")
        xr_ = x_from_dram.rearrange("k p d -> p k d")
        with tc.tile_pool(name=f"vw_{pname}", bufs=1) as vwp, \
                tc.tile_pool(name=f"vx_{pname}", bufs=2) as xvp:
            wts = {}
            for vj in range(NVJ):
                wt = vwp.tile([P, KT * VC], BF, tag=f"v{vj}",
                              name=f"vw_{pname}_{vj}")
                nc.sync.dma_start(
                    out=wt[:], in_=wr[:, :, vj * VC:(vj + 1) * VC])
                for ki in range(KT):
                    wts[(vj, ki)] = wt[:, ki * VC:(ki + 1) * VC]
            for kt in range(nkt):
                xt = xvp.tile([P, KT * P], BF, tag="x",
                              name=f"vx_{pname}")
                nc.sync.dma_start(
                    out=xt[:], in_=xr_[:, :, kt * P:(kt + 1) * P])
                xc = [xt[:, ki * P:(ki + 1) * P] for ki in range(KT)]
                for vj in range(NVJ):
                    ps = acc.tile([P, VC], F32, tag="acc", name="v_ps")
                    for ki in range(KT):
                        nc.tensor.matmul(
                            ps[:], lhsT=xc[ki], rhs=wts[(vj, ki)],
                            start=(ki == 0), stop=(ki == KT - 1))
                        if ki == 3:
                            yield 4
                    evict(kt, vj, ps)
                    yield 4

    def ev_v(kt, vj, ps):
        vsl = Vst[kt][:].rearrange("p (h c) -> p h c", c=HD + 1)
        if vj == 0:
            nc.vector.memset(vsl[:, :, HD:HD + 1], 1.0)
        nc.vector.tensor_add(
            vsl[:, (VC // HD) * vj:(VC // HD) * (vj + 1), 0:HD],
            ps[:], vb_bc[:, vj * VC:(vj + 1) * VC])

    w_vr = dram("wv").rearrange("k p d -> p k d")
    with tc.tile_pool(name="vw_s", bufs=1) as vwp_s:
        for vj in range(NVJ):
            wt_all = vwp_s.tile([P, KT * VC], BF, tag="w", name="vw_s")
            nc.sync.dma_start(
                out=wt_all[:], in_=w_vr[:, :, vj * VC:(vj + 1) * VC])
            wts_s = [wt_all[:, ki * VC:(ki + 1) * VC] for ki in range(KT)]
            for kt in range(NKT):
                ps = acc.tile([P, VC], F32, tag="acc", name="v_ps")
                for ki in range(KT):
                    nc.tensor.matmul(
                        ps[:], lhsT=xdT[ki][:, kt * P:(kt + 1) * P],
                        rhs=wts_s[ki][:],
                        start=(ki == 0), stop=(ki == KT - 1))
                ev_v(kt, vj, ps)
    es_x.close()

    # ---- self-attn q0, woven with cross-K and cross-V projections ----
    def ev_kc(fj, tci, C, ps):
        nc.vector.tensor_scalar_add(
            out=KTc[fj][:, tci * C:(tci + 1) * C], in0=ps[:],
            scalar1=bkc[fj][:])

    def kc_gen():
        """Cross-K projection streaming xeT chunks from DRAM (tci-outer
        so the x chunk is reused across all output groups)."""
        C = 512
        wr = dram("wkc").rearrange("k p d -> p k d")
        xr_ = dram("xeT").rearrange("k p d -> p k d")
        with tc.tile_pool(name="xcp", bufs=2) as xcp, \
                tc.tile_pool(name="wp_wkc", bufs=2) as wp:
            for tci in range(S // C):
                xt = xcp.tile([P, KT * C], BF, tag="x", name="xe_all")
                nc.sync.dma_start(
                    out=xt[:], in_=xr_[:, :, tci * C:(tci + 1) * C])
                xc = [xt[:, ki * C:(ki + 1) * C] for ki in range(KT)]
                for fg in range(HP // ACCG):
                    js = [fg * ACCG, fg * ACCG + 1]
                    cw = ACCG * P
                    wt_all = wp.tile([P, KT * cw], BF, tag="w",
                                     name="w_wkc")
                    nc.sync.dma_start(
                        out=wt_all[:],
                        in_=wr[:, :, js[0] * P:(js[-1] + 1) * P])
                    wts = [wt_all[:, ki * cw:(ki + 1) * cw]
                           for ki in range(KT)]
                    ps = [acc.tile([P, C], F32, tag="acc", name="acc_ps")
                          for _ in js]
                    for ki in range(KT):
                        for jj in range(len(js)):
                            nc.tensor.matmul(
                                ps[jj][:],
                                lhsT=wts[ki][:, jj * P:(jj + 1) * P],
                                rhs=xc[ki],
                                start=(ki == 0), stop=(ki == KT - 1))
                        if ki < KT - 1:
                            yield len(js)
                    for jj, fj in enumerate(js):
                        ev_kc(fj, tci, C, ps[jj])
                    yield len(js)

    def ev_vc_spill(kt, vj, ps):
        st = stagep.tile([P, VC + VC // HD], BF, tag="vcst", name="vcst")
        stv = st[:].rearrange("p (h c) -> p h c", c=HD + 1)
        nc.vector.memset(stv[:, :, HD:HD + 1], 1.0)
        nc.vector.tensor_add(stv[:, :, 0:HD], ps[:],
                             vcb_bc[:, vj * VC:(vj + 1) * VC])
        nh = VC // HD
        dst = dram("vc_spill")[kt][:, vj * nh * (HD + 1):
                                   (vj + 1) * nh * (HD + 1)]
        nc.gpsimd.dma_start(out=dst, in_=st[:])

    fillers.append(kc_gen())
    fillers.append(v_gen(dram("xeT"), "wvc", NSK, vcb_bc,
                           ev_vc_spill, "c"))

    attention_chunk(KTt, QTt, Vst, NKT, expp, "sa", 0, bounce_idx=0)

    # ---- self-attn q1, woven with cross-K/V leftovers ----
    def ev_o1(fj, tci, C, ps):
        xr = stream2.tile([P, C], BF, tag="xr_s", name="xr_s")
        nc.sync.dma_start(out=xr[:],
                          in_=dram("xres")[fj][:, tci * C:(tci + 1) * C])
        nc.vector.scalar_tensor_tensor(
            out=z1b[fj][:, tci * C:(tci + 1) * C], in0=ps[:],
            scalar=bo1[fj][:], in1=xr[:], op0=ALU.add, op1=ALU.add)

    attention_chunk(KTt, QTt, Vst, NKT, expp, "sa", 1, bounce_idx=0)

    # ---- bridge 1: drain; LN1 chunk 0; start cross Q; V reload ----
    drain()
    es_exp.close()
    es_vst.close()

    es_expc = ExitStack()
    expc = es_expc.enter_context(
        tc.tile_pool(name="expc", bufs=3, side="right"))
    es_vsc = ExitStack()
    pvsc = es_vsc.enter_context(
        tc.tile_pool(name="pvsc", bufs=1, side="right"))
    Vsc = [pvsc.tile([P, H * (HD + 1)], BF, tag=f"Vsc{k}", name=f"Vsc{k}")
           for k in range(NSK)]
    for k in range(NSK):
        nc.sync.dma_start(out=Vsc[k][:], in_=dram("vc_spill")[k])

    run_gen(projT_gen("wo1", QTt, D, TL, ev_o1, tcis=[0]))
    layer_norm_chunk(z1b, g1, be1, 0, out_bf=res_bf,
                     spill=[dram("res1_spill")[k] for k in range(KT)])

    def ev_qc(fj, tci, C, ps):
        nc.vector.tensor_scalar_add(
            out=QTc[fj][:, tci * C:(tci + 1) * C], in0=ps[:],
            scalar1=bqc[fj][:])

    wqc0 = projT_gen("wqc", res_bf, D, TL, ev_qc, tcis=[0])
    # first output group dense (cross-attn jt0/jt1 need it immediately)
    budget = 2 * KT
    while budget > 0:
        budget -= next(wqc0)
    fillers.append(wqc0)

    # ---- cross-attn q0, woven with wo1-c1, LN1-c1, wqc-c1 ----
    def ln1c1_gen():
        layer_norm_chunk(z1b, g1, be1, 1, out_bf=res_bf,
                         spill=[dram("res1_spill")[k] for k in range(KT)])
        yield 2 * KT

    fillers.append(projT_gen("wo1", QTt, D, TL, ev_o1, tcis=[1],
                             pname="wp_wo1b"))
    fillers.append(ln1c1_gen())
    fillers.append(projT_gen("wqc", res_bf, D, TL, ev_qc, tcis=[1],
                             pname="wp_wqcb"))

    attention_chunk(KTc, QTc, Vsc, NSK, expc, "ca", 0, bounce_idx=1)

    # ---- bridge 2: drain; release self K/Q ----
    drain()
    es_kq.close()

    # ---- cross-attn q1, woven with cross out-proj chunk 0 ----
    def ev_o2(fj, tci, C, ps):
        xr = stream2.tile([P, C], BF, tag="xr_s", name="xr_s")
        nc.sync.dma_start(out=xr[:],
                          in_=dram("res1_spill")[fj][:, tci * C:(tci + 1) * C])
        nc.vector.scalar_tensor_tensor(
            out=z2b[fj][:, tci * C:(tci + 1) * C], in0=ps[:],
            scalar=bo2[fj][:], in1=xr[:], op0=ALU.add, op1=ALU.add)

    attention_chunk(KTc, QTc, Vsc, NSK, expc, "ca", 1, bounce_idx=1)

    # ---- tail: wo2, LN2, FFN, LN3 (dense) ----
    drain()
    es_vsc.close()
    es_expc.close()

    run_gen(projT_gen("wo2", QTc, D, TL, ev_o2, tcis=[0]))
    layer_norm_chunk(z2b, g2, be2, 0, out_bf=res2_bf,
                     spill=[dram("res2_spill")[k] for k in range(KT)])
    run_gen(projT_gen("wo2", QTc, D, TL, ev_o2, tcis=[1], pname="wp_wo2b"))
    layer_norm_chunk(z2b, g2, be2, 1, out_bf=res2_bf,
                     spill=[dram("res2_spill")[k] for k in range(KT)])
    es_kc.close()

    es_ff = ExitStack()
    pff = es_ff.enter_context(tc.tile_pool(name="pff", bufs=1))
    hT = [pff.tile([P, TL], BF, tag=f"hT{f}", name=f"hT{f}")
          for f in range(FT)]
    z3b = [pff.tile([P, TL], BF, tag=f"z3b{k}", name=f"z3b{k}")
           for k in range(KT)]

    def ev_f1(fj, tci, C, ps):
        nc.vector.tensor_scalar(
            out=hT[fj][:, tci * C:(tci + 1) * C], in0=ps[:],
            scalar1=b1f[fj][:], scalar2=0.0, op0=ALU.add, op1=ALU.max)

    def ev_f2(fj, tci, C, ps):
        xr = stream2.tile([P, C], BF, tag="xr_s", name="xr_s")
        nc.sync.dma_start(out=xr[:],
                          in_=dram("res2_spill")[fj][:, tci * C:(tci + 1) * C])
        nc.vector.scalar_tensor_tensor(
            out=z3b[fj][:, tci * C:(tci + 1) * C], in0=ps[:],
            scalar=b2f[fj][:], in1=xr[:], op0=ALU.add, op1=ALU.add)

    run_gen(projT_gen("w1", res2_bf, DFF, TL, ev_f1, tcis=[0], wp_bufs=3))
    run_gen(projT_gen("w1", res2_bf, DFF, TL, ev_f1, tcis=[1], wp_bufs=3,
                      pname="wp_w1b"))
    es_qc.close()
    run_gen(projT_gen("w2", hT, D, TL, ev_f2, tcis=[0], kt_in=FT,
                      wp_bufs=1))
    layer_norm_chunk(z3b, g3, be3, 0, out_bf=z3b,
                     out_dram=[dram("outT")[k] for k in range(KT)])
    run_gen(projT_gen("w2", hT, D, TL, ev_f2, tcis=[1], kt_in=FT,
                      wp_bufs=1, pname="wp_w2b"))
    layer_norm_chunk(z3b, g3, be3, 1, out_bf=z3b,
                     out_dram=[dram("outT")[k] for k in range(KT)])
    es_ff.close()
    ctx.close()


# ----------------------------------------------------------------------------
# host glue
# ----------------------------------------------------------------------------

def _to_bf(a):
    return np.ascontiguousarray(np.asarray(a).astype(ml_dtypes.bfloat16))


def _to_f32(a):
    return np.ascontiguousarray(np.asarray(a).astype(np.float32))


def _prep_weights(inp, D, H, DFF):
    KT = D // P

    def tile_w(w):
        return _to_bf(w.reshape(w.shape[0] // P, P, w.shape[1]))

    hidx = np.arange(H)[:, None] * 3 * HD + np.arange(HD)[None, :]
    perm_q = hidx.ravel()
    perm_k = (hidx + HD).ravel()
    perm_v = (hidx + 2 * HD).ravel()
    qkv_w, qkv_b = inp["qkv_w"], inp["qkv_b"]
    kv_w, kv_b = inp["kv_w"], inp["kv_b"]
    h2 = np.arange(H)[:, None] * 2 * HD + np.arange(HD)[None, :]
    perm_kc = h2.ravel()
    perm_vc = (h2 + HD).ravel()

    def cols(b):
        return np.asarray(b).reshape(-1, P).T

    ppb = np.concatenate([
        cols(qkv_b[perm_q]), cols(qkv_b[perm_k]),
        cols(inp["sa_o_b"]), cols(inp["q_b"]),
        cols(kv_b[perm_kc]), cols(inp["ca_o_b"]),
        cols(inp["ff_b2"]),
        cols(inp["g1"]), cols(inp["be1"]),
        cols(inp["g2"]), cols(inp["be2"]),
        cols(inp["g3"]), cols(inp["be3"]),
        cols(inp["ff_b1"]),
    ], axis=1)

    return dict(
        wq=tile_w(qkv_w[:, perm_q]), wk=tile_w(qkv_w[:, perm_k]),
        wv=tile_w(qkv_w[:, perm_v]),
        bv_row=_to_f32(qkv_b[perm_v].reshape(1, D)),
        wo1=tile_w(inp["sa_o_w"]),
        wqc=tile_w(inp["q_w"]),
        wkc=tile_w(kv_w[:, perm_kc]),
        wvc=tile_w(kv_w[:, perm_vc]),
        bvc_row=_to_f32(kv_b[perm_vc].reshape(1, D)),
        wo2=tile_w(inp["ca_o_w"]),
        w1=tile_w(inp["ff_w1"]),
        w2=tile_w(inp["ff_w2"]),
        ppb=_to_f32(ppb),
    )


def make_in_maps(inputs, n_cores=8):
    inp = {k: np.asarray(v) for k, v in inputs.items()}
    B, T, D = inp["x_dec"].shape
    S = inp["x_enc"].shape[1]
    DFF = inp["ff_w1"].shape[1]
    H = D // HD
    KT = D // P
    halves = n_cores // B
    TL = T // halves
    shared = _prep_weights(inp, D, H, DFF)
    in_maps = []
    for c in range(n_cores):
        b, half = c // halves, c % halves
        xd = inp["x_dec"][b]
        xe = inp["x_enc"][b]
        own = xd[half * TL:(half + 1) * TL]
        # rotate so the own-query rows sit at columns [0, TL) of xdT;
        # self-attn keys/values permute identically (softmax-invariant).
        xd_rot = np.concatenate([own, xd[:half * TL], xd[(half + 1) * TL:]])
        m = dict(shared)
        m["xdT"] = _to_bf(xd_rot.T.reshape(KT, P, T))
        m["xres"] = _to_bf(own.T.reshape(KT, P, TL))
        m["xeT"] = _to_bf(xe.T.reshape(KT, P, S))
        in_maps.append(m)
    return in_maps, (B, T, D, TL, S, DFF, H, halves)


def assemble_output(results, meta):
    B, T, D, TL, S, DFF, H, halves = meta
    out = np.empty((B, T, D), np.float32)
    for c, r in enumerate(results):
        b, half = c // halves, c % halves
        yT = np.asarray(r["outT"]).astype(np.float32).reshape(D, TL)
        out[b, half * TL:(half + 1) * TL] = yT.T
    return out


def kernel(**inputs):
    in_maps, meta = make_in_maps(inputs)
    B, T, D, TL, S, DFF, H, halves = meta
    nc = build_program(D=D, H=H, T=T, TL=TL, S=S, DFF=DFF)
    res = run_bass_kernel_spmd(nc, in_maps, core_ids=list(range(len(in_maps))))
    return assemble_output(res.results, meta)
